# revision 1
# baseline (speedup 1.0000x reference)
"""KVGather kernel for Trainium2 (8 NeuronCores).

Problem: r_idx (4, 64, 16) int values in [0, 64); kv (4, 64, 49, 512) f32.
Output (4, 64, 16, 49, 512) f32 = kv[b, r_idx[b, p, k]] for each (b, p, k).

Strategy
--------
Pure data movement: each gathered region kv[b, r] is a contiguous
49*512*4 = 100,352-byte block; the output is 392 MiB of such blocks.

Sharding: 8 shards = (batch b: 4) x (p2 half: 2). Each core owns the full
kv[b] (6.4 MB) and produces output rows for its 32 p2 positions
(512 output regions = 51.4 MB).

Per core:
  1. DMA kv[b] into SBUF once, laid out as [128 partitions x 12544 f32]
     (partition 2r+h = half h of region r; this is the natural contiguous
     reshape of kv[b]).
  2. Invert r_idx on the host: for each region r, the list of output
     regions that reference it. Ship as an int32 table [128, M] of
     destination half-row indices (M = max multiplicity), padded with an
     out-of-bounds sentinel.
  3. For m in range(M): one gpsimd indirect (scatter) DMA writes SBUF
     partition p -> output half-row table[p, m]. OOB sentinel rows are
     skipped by the hardware bounds check.

So each kv byte is read from HBM exactly once, and the 51.4 MB output
shard is written with ~M large scatter DMAs instead of 512 small ones.
"""

import numpy as np

B, P2, TOPK, W2, C_KV = 4, 64, 16, 49, 512
N_CORES = 8
HALF_P2 = P2 // 2  # 32 p2 rows per core
N_OUT_REG = HALF_P2 * TOPK  # 512 output regions per core
N_OUT_ROWS = N_OUT_REG * 2  # 1024 half-region rows per core
D = W2 * C_KV // 2  # 12544 f32 per half-region row
OOB_SENTINEL = 0x7FFF  # any value > N_OUT_ROWS - 1


def _build_program(m_slots: int, repeats: int = 1, split: int = 1):
    """repeats > 1 replays the whole body; used only for benchmarking
    (marginal time per repeat isolates kernel time from dispatch/transfer
    overhead).

    split = number of partition groups the kv load + scatters are divided
    into; group g's scatters can start as soon as group g's slice of kv has
    landed, hiding most of the load latency behind the first writes."""
    import concourse.bass as bass
    import concourse.mybir as mybir

    assert 128 % split == 0
    pg = 128 // split  # partitions per group

    nc = bass.Bass()
    kv_in = nc.dram_tensor("kv", [128, D], mybir.dt.float32, kind="ExternalInput")
    idx_in = nc.dram_tensor(
        "idx", [128, m_slots], mybir.dt.int32, kind="ExternalInput"
    )
    out = nc.dram_tensor(
        "out", [N_OUT_ROWS, D], mybir.dt.float32, kind="ExternalOutput"
    )

    import contextlib

    with contextlib.ExitStack() as ctx:
        kv_sb = ctx.enter_context(nc.sbuf_tensor([128, D], mybir.dt.float32))
        idx_sb = ctx.enter_context(nc.sbuf_tensor([128, m_slots], mybir.dt.int32))
        dma_sem = ctx.enter_context(nc.semaphore("dma_sem"))
        load_sems = [
            ctx.enter_context(nc.semaphore(f"load_sem{s}")) for s in range(split)
        ]
        block = ctx.enter_context(nc.Block())

        @block.gpsimd
        def _(g):
            with g.register("bc") as bc_reg:
                g.reg_mov(bc_reg, N_OUT_ROWS - 1)
                sem = 0
                for rep in range(repeats):
                    g.dma_start(idx_sb[:], idx_in[:]).then_inc(dma_sem, 16)
                    for s in range(split):
                        lo, hi = s * pg, (s + 1) * pg
                        g.dma_start(kv_sb[lo:hi, :], kv_in[lo:hi, :]).then_inc(
                            load_sems[s], 16
                        )
                    sem += 16
                    g.wait_ge(dma_sem, sem)  # idx loaded
                    for s in range(split):
                        lo, hi = s * pg, (s + 1) * pg
                        g.wait_ge(load_sems[s], 16 * (rep + 1))
                        for m in range(m_slots):
                            g.indirect_dma_start(
                                out=out[:],
                                out_offset=bass.IndirectOffsetOnAxis(
                                    ap=idx_sb[lo:hi, m : m + 1], axis=0
                                ),
                                in_=kv_sb[lo:hi, :],
                                in_offset=None,
                                bounds_check=bc_reg,
                                oob_is_err=False,
                            ).then_inc(dma_sem, 16)
                        sem += 16 * m_slots
                    g.wait_ge(dma_sem, sem)

    return nc


def _make_tables(r_idx: np.ndarray):
    """Per-core inverse-index tables.

    Returns (m_slots, list of per-core [128, m_slots] int32 tables)."""
    per_core_lists = []
    m_slots = 1
    for c in range(N_CORES):
        b, h = divmod(c, 2)
        local = (
            np.asarray(r_idx[b, h * HALF_P2 : (h + 1) * HALF_P2, :])
            .reshape(-1)
            .astype(np.int64)
        )
        lists = [[] for _ in range(P2)]
        for j, r in enumerate(local):
            lists[int(r)].append(j)
        m_slots = max(m_slots, max(len(l) for l in lists))
        per_core_lists.append(lists)

    tables = []
    for lists in per_core_lists:
        table = np.full((128, m_slots), OOB_SENTINEL, dtype=np.int32)
        for r, l in enumerate(lists):
            for m, j in enumerate(l):
                table[2 * r, m] = 2 * j
                table[2 * r + 1, m] = 2 * j + 1
        tables.append(table)
    return m_slots, tables


def _run(r_idx: np.ndarray, kv: np.ndarray, trace: bool = False):
    from concourse.bass_utils import run_bass_kernel_spmd

    m_slots, tables = _make_tables(r_idx)
    nc = _build_program(m_slots)

    in_maps = []
    for c in range(N_CORES):
        b = c // 2
        in_maps.append(
            {
                "kv": np.ascontiguousarray(kv[b]).reshape(128, D),
                "idx": tables[c],
            }
        )

    res = run_bass_kernel_spmd(
        nc, in_maps, core_ids=list(range(N_CORES)), trace=trace
    )

    out = np.empty((B, P2, TOPK, W2, C_KV), dtype=np.float32)
    for c in range(N_CORES):
        b, h = divmod(c, 2)
        out[b, h * HALF_P2 : (h + 1) * HALF_P2] = res.results[c]["out"].reshape(
            HALF_P2, TOPK, W2, C_KV
        )
    return out, res


def kernel(r_idx: np.ndarray, kv: np.ndarray) -> np.ndarray:
    r_idx = np.asarray(r_idx)
    kv = np.asarray(kv, dtype=np.float32)
    out, _ = _run(r_idx, kv, trace=False)
    return out



# revision 3
# speedup vs baseline: 324.0859x; 324.0859x over previous
"""KVGather kernel for Trainium2 (8 NeuronCores).

Problem: r_idx (4, 64, 16) int values in [0, 64); kv (4, 64, 49, 512) f32.
Output (4, 64, 16, 49, 512) f32 = kv[b, r_idx[b, p, k]] for each (b, p, k).

Strategy
--------
Pure data movement: each gathered region kv[b, r] is a contiguous
49*512*4 = 100,352-byte block; the output is 392 MiB of such blocks.

Sharding: 8 shards = (batch b: 4) x (p2 half: 2). Each core owns the full
kv[b] (6.4 MB) and produces output rows for its 32 p2 positions
(512 output regions = 51.4 MB).

Per core:
  1. DMA kv[b] into SBUF once as [128 partitions x 12544 f32] half-region
     rows, HOST-PERMUTED so that the 16 SDMA engines (each hard-wired to a
     fixed set of 8 partitions) carry equal numbers of output rows.
  2. Invert r_idx on the host: for each half-region, the list of output
     half-rows that reference it. Split each list into chunks, one chunk
     per partition (128 chunks), assigned to partitions by longest-
     processing-time so per-engine totals are ~equal (64/engine vs up to
     102 unbalanced). Ship as an int32 table [128, M] of destination
     half-row indices, padded with an out-of-bounds sentinel.
  3. For m in range(M): one gpsimd indirect (scatter) DMA writes SBUF
     partition p -> output half-row table[p, m]. OOB sentinel rows are
     skipped by the hardware bounds check.

So each kv byte is read from HBM exactly once, the 51.4 MB output shard
is written with M large scatter DMAs, and every engine moves the same
number of 50 KB rows.
"""

import heapq

import numpy as np

B, P2, TOPK, W2, C_KV = 4, 64, 16, 49, 512
N_CORES = 8
HALF_P2 = P2 // 2  # 32 p2 rows per core
N_OUT_REG = HALF_P2 * TOPK  # 512 output regions per core
N_OUT_ROWS = N_OUT_REG * 2  # 1024 half-region rows per core
D = W2 * C_KV // 2  # 12544 f32 per half-region row
OOB_SENTINEL = 0x7FFF  # any value > N_OUT_ROWS - 1

# SDMA engine serving partition p (the port swizzle): engine
# 2*((p%32)//4) + p//64 reads partitions {4k..4k+3, 4k+32..4k+35}.
ENGINE_OF_P = np.array([2 * ((p % 32) // 4) + (p // 64) for p in range(128)])


def _build_program(m_slots: int, repeats: int = 1, split: int = 1):
    """repeats > 1 replays the whole body; used only for benchmarking
    (marginal time per repeat isolates kernel time from dispatch/transfer
    overhead).

    split = number of partition groups the kv load + scatters are divided
    into; group g's scatters can start as soon as group g's slice of kv has
    landed, hiding most of the load latency behind the first writes."""
    import concourse.bass as bass
    import concourse.mybir as mybir

    assert 128 % split == 0
    pg = 128 // split  # partitions per group

    nc = bass.Bass()
    kv_in = nc.dram_tensor("kv", [128, D], mybir.dt.float32, kind="ExternalInput")
    idx_in = nc.dram_tensor(
        "idx", [128, m_slots], mybir.dt.int32, kind="ExternalInput"
    )
    out = nc.dram_tensor(
        "out", [N_OUT_ROWS, D], mybir.dt.float32, kind="ExternalOutput"
    )

    import contextlib

    with contextlib.ExitStack() as ctx:
        kv_sb = ctx.enter_context(nc.sbuf_tensor([128, D], mybir.dt.float32))
        idx_sb = ctx.enter_context(nc.sbuf_tensor([128, m_slots], mybir.dt.int32))
        dma_sem = ctx.enter_context(nc.semaphore("dma_sem"))
        load_sems = [
            ctx.enter_context(nc.semaphore(f"load_sem{s}")) for s in range(split)
        ]
        block = ctx.enter_context(nc.Block())

        @block.gpsimd
        def _(g):
            with g.register("bc") as bc_reg:
                g.reg_mov(bc_reg, N_OUT_ROWS - 1)
                sem = 0
                for rep in range(repeats):
                    g.dma_start(idx_sb[:], idx_in[:]).then_inc(dma_sem, 16)
                    for s in range(split):
                        lo, hi = s * pg, (s + 1) * pg
                        g.dma_start(kv_sb[lo:hi, :], kv_in[lo:hi, :]).then_inc(
                            load_sems[s], 16
                        )
                    sem += 16
                    g.wait_ge(dma_sem, sem)  # idx loaded
                    for s in range(split):
                        lo, hi = s * pg, (s + 1) * pg
                        g.wait_ge(load_sems[s], 16 * (rep + 1))
                        for m in range(m_slots):
                            g.indirect_dma_start(
                                out=out[:],
                                out_offset=bass.IndirectOffsetOnAxis(
                                    ap=idx_sb[lo:hi, m : m + 1], axis=0
                                ),
                                in_=kv_sb[lo:hi, :],
                                in_offset=None,
                                bounds_check=bc_reg,
                                oob_is_err=False,
                            ).then_inc(dma_sem, 16)
                        sem += 16 * m_slots
                    g.wait_ge(dma_sem, sem)

    return nc


def _make_core_plan(local_ridx):
    """local_ridx: (HALF_P2, TOPK) in [0, P2). Returns (perm, table):
    perm[p] = source half-row (2r+h) whose data partition p holds;
    table[p, m] = destination half-row for partition p's m-th copy."""
    flat = local_ridx.reshape(-1)
    dest = [[] for _ in range(2 * P2)]  # dest half-rows per source half-row
    for j, r in enumerate(flat):
        dest[2 * int(r)].append(2 * j)
        dest[2 * int(r) + 1].append(2 * j + 1)

    # one chunk per non-empty source half-row; split the largest until 128
    heap = [(-len(d), row, d) for row, d in enumerate(dest) if d]
    heapq.heapify(heap)
    n = len(heap)
    while n < 128:
        negc, row, d = heapq.heappop(heap)
        c = -negc
        if c <= 1:
            heapq.heappush(heap, (negc, row, d))
            break
        a, b = d[: (c + 1) // 2], d[(c + 1) // 2 :]
        heapq.heappush(heap, (-len(a), row, a))
        heapq.heappush(heap, (-len(b), row, b))
        n += 1
    chunks = [(-negc, row, d) for negc, row, d in heap]
    while len(chunks) < 128:  # degenerate inputs: pad with empty chunks
        chunks.append((0, 0, []))

    # LPT: assign chunks to the 16 engines (8 partition slots each) so
    # per-engine copy totals are ~equal
    chunks.sort(key=lambda t: -t[0])
    bin_load = np.zeros(16, dtype=np.int64)
    bin_slots = [[] for _ in range(16)]
    for c, row, d in chunks:
        order = np.lexsort((np.arange(16), bin_load))
        for e in order:
            if len(bin_slots[e]) < 8:
                bin_slots[e].append((row, d))
                bin_load[e] += c
                break

    perm = np.zeros(128, dtype=np.int64)
    m_slots = max(len(d) for slots in bin_slots for _, d in slots)
    table = np.full((128, m_slots), OOB_SENTINEL, dtype=np.int32)
    for e in range(16):
        parts = np.where(ENGINE_OF_P == e)[0]
        for (row, d), p in zip(bin_slots[e], parts):
            perm[p] = row
            table[p, : len(d)] = d
    return perm, table


def _make_plan(r_idx: np.ndarray):
    """Per-core balanced scatter plans.

    Returns (m_slots, perms, tables): perms[c] is the [128] source-row
    permutation for core c's kv SBUF layout, tables[c] the [128, m_slots]
    int32 destination table (sentinel-padded)."""
    perms, tables = [], []
    for c in range(N_CORES):
        b, h = divmod(c, 2)
        perm, table = _make_core_plan(
            np.asarray(r_idx[b, h * HALF_P2 : (h + 1) * HALF_P2, :])
        )
        perms.append(perm)
        tables.append(table)
    m_slots = max(t.shape[1] for t in tables)
    tables = [
        np.pad(t, ((0, 0), (0, m_slots - t.shape[1])), constant_values=OOB_SENTINEL)
        for t in tables
    ]
    return m_slots, perms, tables


def _in_maps(r_idx: np.ndarray, kv: np.ndarray):
    m_slots, perms, tables = _make_plan(r_idx)
    maps = []
    for c in range(N_CORES):
        b = c // 2
        rows = np.ascontiguousarray(kv[b]).reshape(128, D)
        maps.append(
            {"kv": np.ascontiguousarray(rows[perms[c]]), "idx": tables[c]}
        )
    return m_slots, maps


def _run(r_idx: np.ndarray, kv: np.ndarray, trace: bool = False):
    from concourse.bass_utils import run_bass_kernel_spmd

    m_slots, in_maps = _in_maps(r_idx, kv)
    nc = _build_program(m_slots)

    res = run_bass_kernel_spmd(
        nc, in_maps, core_ids=list(range(N_CORES)), trace=trace
    )

    out = np.empty((B, P2, TOPK, W2, C_KV), dtype=np.float32)
    for c in range(N_CORES):
        b, h = divmod(c, 2)
        out[b, h * HALF_P2 : (h + 1) * HALF_P2] = res.results[c]["out"].reshape(
            HALF_P2, TOPK, W2, C_KV
        )
    return out, res


def kernel(r_idx: np.ndarray, kv: np.ndarray) -> np.ndarray:
    r_idx = np.asarray(r_idx)
    kv = np.asarray(kv, dtype=np.float32)
    out, _ = _run(r_idx, kv, trace=False)
    return out
